# revision 2
# baseline (speedup 1.0000x reference)
"""DJMGNN (NNConv/GraphNorm GNN) Trainium2 kernel, 8-core SPMD — v2.

Sharding: nodes range-sharded N/8 per core, then RELABELED per core so each
128-node window has a balanced edge count (fewer padded edge blocks).
Edges live with their dst owner, grouped into 128-dst windows.

Per layer:
  - h shard staged to DRAM, AllGathered into a global table, per-block
    indirect-DMA gather of h[src].
  - edge MLP on PE (attrT stationary, bias as ones-row) -> PSUM.
  - relu+mult split ACT/DVE (3/4 blocks: ACT relu to bf16 + DVE 2x mult;
    1/4 blocks fused scalar_tensor_tensor on DVE from PSUM).
  - scatter matmul folds the i-reduction: moving tmp viewed i-major
    (strided) + stride-0 PSUM out AP accumulates 32 i-columns into one
    agg[u, o] column. conv = agg + h@rootW + b accumulates in the same
    PSUM bank (bias ones-row matmul + row-tiled hT stationary matmul).
  - GraphNorm stats accumulate in PSUM across windows (masked column
    stationary), one tiny AllReduce, then batched DVE node ops.
  - transition = h@trans_top (precomputed at layer start, overlapping the
    AllGather) + hc@trans_bot (row-tiled, accumulated in PSUM).
"""

import sys

if "/opt/trn_rl_repo" not in sys.path:
    sys.path.insert(0, "/opt/trn_rl_repo")

import heapq

import numpy as np
import ml_dtypes

import concourse.bass as bass
import concourse.bacc as bacc
import concourse.mybir as mybir
import concourse.tile as tile

mdt = mybir.dt
AF = mybir.ActivationFunctionType
ALU = mybir.AluOpType

NCORES = 8
EPS = 1e-5
# Fold the i-reduction into the scatter matmul via a stride-0 PSUM out AP.
# HW handles it (probe-verified); CoreSim cannot model it, so sim runs set
# this False to fall back to a plain [128, HH] accumulate + DVE reduce.
FOLD = True


# ---------------------------------------------------------------- host prep


def _balance_groups(dst_local, n_edges_of, shard, shard_pad, nwin):
    """Assign the shard's node slots to nwin groups of 128, balancing the
    per-group edge counts. Returns perm (old local -> new slot) and the
    per-group edge counts (sorted desc over groups)."""
    order = np.argsort(-n_edges_of, kind="stable")
    heap = [(0, g) for g in range(nwin)]
    heapq.heapify(heap)
    gcount = np.zeros(nwin, np.int64)
    gnodes = [[] for _ in range(nwin)]
    for v in order:
        while True:
            cnt, g = heapq.heappop(heap)
            if len(gnodes[g]) < 128:
                break
        gnodes[g].append(v)
        gcount[g] += n_edges_of[v]
        heapq.heappush(heap, (int(gcount[g]), g))
    # order groups by edge count desc so heavy groups align across cores
    grank = np.argsort(-gcount, kind="stable")
    perm = np.full(shard_pad, -1, np.int64)
    counts = np.zeros(nwin, np.int64)
    pads = list(range(shard, shard_pad))
    for w, g in enumerate(grank):
        nodes = gnodes[g]
        for j, v in enumerate(nodes):
            perm[v] = w * 128 + j
        for j in range(len(nodes), 128):
            perm[pads.pop()] = w * 128 + j
        counts[w] = gcount[g]
    return perm, counts


def _shard_graph(edge_index, n_nodes, shard, shard_pad, blk=128):
    src = edge_index[0].astype(np.int64)
    dst = edge_index[1].astype(np.int64)
    owner = dst // shard
    dst_local = dst - owner * shard
    nwin = shard_pad // 128

    perms = []
    all_counts = np.zeros((NCORES, nwin), np.int64)
    for c in range(NCORES):
        deg = np.bincount(dst_local[owner == c], minlength=shard)
        perm, counts = _balance_groups(dst_local, deg, shard, shard_pad, nwin)
        perms.append(perm)
        all_counts[c] = counts

    bw = [max(1, int(np.max((all_counts[:, w] + blk - 1) // blk)))
          for w in range(nwin)]
    block_win = []
    for w in range(nwin):
        block_win += [w] * bw[w]
    nblocks = len(block_win)

    # per-core edge slots
    eslot = np.full((NCORES, nblocks, blk), -1, dtype=np.int64)
    new_dst = np.empty_like(dst_local)
    for c in range(NCORES):
        m = owner == c
        new_dst[m] = perms[c][dst_local[m]]
    wid = new_dst // 128
    for c in range(NCORES):
        b0 = np.cumsum([0] + bw[:-1])
        fill = np.zeros(nwin, np.int64)
        idxs = np.nonzero(owner == c)[0]
        for e in idxs:
            w = wid[e]
            j = fill[w]
            eslot[c, b0[w] + j // blk, j % blk] = e
            fill[w] += 1
    return eslot, block_win, bw, nwin, perms, new_dst


def prep_inputs(inputs):
    x = np.asarray(inputs["x"], np.float32)
    edge_attr = np.asarray(inputs["edge_attr"], np.float32)
    edge_index = np.asarray(inputs["edge_index"])
    N, IN = x.shape
    E, EA = edge_attr.shape
    H = np.asarray(inputs["init_W"]).shape[1]
    L = np.asarray(inputs["edge_mlp_W"]).shape[0]
    T = np.asarray(inputs["final_W"]).shape[1]
    shard = N // NCORES
    shard_pad = ((shard + 127) // 128) * 128
    nub = shard_pad // 128

    eslot, block_win, bw, nwin, perms, new_dst = _shard_graph(
        edge_index, N, shard, shard_pad)
    B = len(block_win)
    src = edge_index[0].astype(np.int64)

    # o-major reorder of edge MLP weights: col j = i*H + o -> o*H + i
    Wm = np.asarray(inputs["edge_mlp_W"], np.float32).reshape(L, EA, H, H)
    Wm = Wm.transpose(0, 1, 3, 2).reshape(L, EA, H * H)
    bm = np.asarray(inputs["edge_mlp_b"], np.float32).reshape(L, H, H)
    bm = bm.transpose(0, 2, 1).reshape(L, H * H)
    W_aug = np.concatenate([Wm, bm[:, None, :]], axis=1)  # [L, EA+1, H*H]
    # zero-pad contraction dim to 128: K<128 matmuls keep the PE HAM clock
    # gate cold (1.2 GHz); full-K runs at 2.4 GHz
    W_pad = np.zeros((L, 128, H * H), np.float32)
    W_pad[:, : EA + 1] = W_aug

    init_aug = np.concatenate(
        [np.asarray(inputs["init_W"], np.float32),
         np.asarray(inputs["init_b"], np.float32)[None, :]], axis=0
    )  # [IN+1, H]

    rootW = np.asarray(inputs["root_W"], np.float32)  # [L, H, H]
    rootb = np.asarray(inputs["root_b"], np.float32)[:, None, :]  # [L, 1, H]
    root33 = np.zeros((L, 128, H), np.float32)
    root33[:, :H] = rootW
    root33[:, H : H + 1] = rootb

    transW = np.asarray(inputs["trans_W"], np.float32)  # [L, 2H, H]
    transb = np.asarray(inputs["trans_b"], np.float32)[:, None, :]  # [L, 1, H]
    transh33 = np.zeros((L, 128, H), np.float32)
    transh33[:, :H] = transW[:, :H] + transW[:, H:]
    transh33[:, H : H + 1] = transb
    tbot = np.zeros((L, 4, 128, H), np.float32)
    for k in range(4):
        tbot[:, k, 32 * k : 32 * k + H] = transW[:, H:]
    tbot = np.ascontiguousarray(tbot.transpose(0, 2, 1, 3))  # [L, 128, 4, H]

    final33 = np.zeros((128, T), np.float32)
    final33[:H] = np.asarray(inputs["final_W"], np.float32)
    final33[H] = np.asarray(inputs["final_b"], np.float32)

    gnms = np.asarray(inputs["gn_ms"], np.float32)
    gn = np.concatenate(
        [np.asarray(inputs["gn_w"], np.float32),
         np.asarray(inputs["gn_b"], np.float32),
         gnms, gnms * (2.0 - gnms)], axis=1
    )[:, None, :]  # [L, 1, 4H]
    fgnms = np.asarray(inputs["fgn_ms"], np.float32)
    fgn = np.concatenate(
        [np.asarray(inputs["fgn_w"], np.float32),
         np.asarray(inputs["fgn_b"], np.float32),
         fgnms, fgnms * (2.0 - fgnms)], axis=0
    )[None, :]  # [1, 4T]

    ident = np.eye(128, dtype=np.float32)
    ones_row = np.ones((1, 128), np.float32)

    in_maps = []
    for c in range(NCORES):
        es = eslot[c]
        valid = es >= 0
        esc = np.where(valid, es, 0)

        attrT_aug = np.zeros((128, B * 128), np.float32)
        attrT_aug[:EA, :] = edge_attr[esc.reshape(-1)].T * valid.reshape(-1)
        attrT_aug[EA, :] = valid.reshape(-1).astype(np.float32)

        sg = src[esc.reshape(-1)]
        sown = sg // shard
        gidx = sown * shard_pad + np.concatenate(
            [perms[int(o)][None, :] for o in range(NCORES)], axis=0
        )[sown, sg - sown * shard]
        gidx = np.where(valid.reshape(-1), gidx, 0).astype(np.int32)
        src_gidx = gidx.reshape(B, 128).T.copy()

        dl = new_dst[esc.reshape(-1)]
        base = np.repeat(np.array([128 * w for w in block_win]), 128)
        dri = np.where(valid.reshape(-1), dl - base, -1).astype(np.int64)
        onehot = np.zeros((B * 128, 128), np.float32)
        vv = dri >= 0
        onehot[np.nonzero(vv)[0], dri[vv]] = 1.0
        # [128(e), B, 128(u)]
        onehot = onehot.reshape(B, 128, 128).transpose(1, 0, 2)

        # x columns in permuted slot order; ones row zero on pad slots
        xs = np.zeros((shard_pad, IN), np.float32)
        inv = np.argsort(perms[c])
        real = inv < shard
        xs[real] = x[c * shard:(c + 1) * shard][inv[real]]
        xT_aug = np.concatenate([xs.T, real[None, :].astype(np.float32)], axis=0)

        mask = real.reshape(nub, 128).T.astype(np.float32).copy() / N  # [128, nub]

        in_maps.append(
            {
                "attrT_aug": np.ascontiguousarray(attrT_aug).astype(ml_dtypes.bfloat16),
                "onehot": np.ascontiguousarray(onehot).astype(ml_dtypes.bfloat16),
                "src_gidx": np.ascontiguousarray(src_gidx),
                "xT_aug": np.ascontiguousarray(xT_aug).astype(ml_dtypes.bfloat16),
                "mask": mask,
                "W_aug": W_pad.astype(ml_dtypes.bfloat16),
                "init_aug": init_aug.astype(ml_dtypes.bfloat16),
                "root33": root33.astype(ml_dtypes.bfloat16),
                "transh33": transh33.astype(ml_dtypes.bfloat16),
                "tbot": tbot.astype(ml_dtypes.bfloat16),
                "final33": final33.astype(ml_dtypes.bfloat16),
                "gn": gn,
                "fgn": fgn,
                "ident": ident,
                "ones_row": ones_row,
            }
        )

    shapes = dict(
        N=N, E=E, IN=IN, H=H, EA=EA, T=T, L=L, shard=shard, shard_pad=shard_pad,
        nub=nub, B=B, block_win=tuple(block_win), bw=tuple(bw), nwin=nwin,
    )
    return in_maps, shapes, perms


# ------------------------------------------------------------- device build


def build_program(s):
    H, EA, IN, T, L = s["H"], s["EA"], s["IN"], s["T"], s["L"]
    B, nub = s["B"], s["nub"]
    shard_pad = s["shard_pad"]
    block_win = s["block_win"]
    HH = H * H
    n_real = s["N"]
    ngrp = nub // 4  # batched-transpose groups (nub assumed %4==0)
    assert nub % 4 == 0

    nc = bacc.Bacc("TRN2", target_bir_lowering=False, debug=False,
                   enable_asserts=False, num_devices=NCORES)

    def din(name, shape, dtype=mdt.float32):
        return nc.dram_tensor(name, shape, dtype, kind="ExternalInput").ap()

    attrT = din("attrT_aug", [128, B * 128], mdt.bfloat16)
    onehot_in = din("onehot", [128, B, 128], mdt.bfloat16)
    src_gidx = din("src_gidx", [128, B], mdt.int32)
    xT_aug = din("xT_aug", [IN + 1, shard_pad], mdt.bfloat16)
    mask_in = din("mask", [128, nub])
    W_in = din("W_aug", [L, 128, HH], mdt.bfloat16)
    init_in = din("init_aug", [IN + 1, H], mdt.bfloat16)
    root_in = din("root33", [L, 128, H], mdt.bfloat16)
    transh_in = din("transh33", [L, 128, H], mdt.bfloat16)
    tbot_in = din("tbot", [L, 128, 4, H], mdt.bfloat16)
    final_in = din("final33", [128, T], mdt.bfloat16)
    gn_in = din("gn", [L, 1, 4 * H])
    fgn_in = din("fgn", [1, 4 * T])
    ident_in = din("ident", [128, 128])
    ones_in = din("ones_row", [1, 128])

    out_dram = nc.dram_tensor("out", [shard_pad, T], mdt.float32,
                              kind="ExternalOutput").ap()

    rg = [list(range(NCORES))]

    with tile.TileContext(nc) as tc:
        with (
            tc.tile_pool(name="const", bufs=1) as cpool,
            tc.tile_pool(name="hbuf", bufs=1) as hpool,
            tc.tile_pool(name="tmp", bufs=8) as tmppool,
            tc.tile_pool(name="rows", bufs=10) as rpool,
            tc.tile_pool(name="ps_pre", bufs=2, space="PSUM") as ps_pre,
            tc.tile_pool(name="ps_aggi", bufs=1, space="PSUM") as ps_aggi,
            tc.tile_pool(name="ps_sm", bufs=2, space="PSUM") as ps_sm,
            tc.tile_pool(name="dram", bufs=1, space="DRAM") as dram,
        ):
            def load(pool, shape, ap, dtype=mdt.float32, tag=None):
                t = pool.tile(shape, dtype, tag=tag)
                nc.sync.dma_start(t[:], ap)
                return t

            attrT_sb = load(cpool, [128, B * 128], attrT[:], mdt.bfloat16, tag="attrT")
            idx_sb = load(cpool, [128, B], src_gidx[:], mdt.int32, tag="sidx")
            xT_sb = load(cpool, [IN + 1, shard_pad], xT_aug[:], mdt.bfloat16, tag="xT")
            mask_sb = load(cpool, [128, nub], mask_in[:], tag="mask")
            init_sb = load(cpool, [IN + 1, H], init_in[:], mdt.bfloat16, tag="initw")
            ident_sb = load(cpool, [128, 128], ident_in[:], tag="ident")
            onesr_sb = load(cpool, [1, 128], ones_in[:], tag="onesr")
            fgn_sb = load(cpool, [1, 4 * T], fgn_in[:], tag="fgn")
            final_sb = load(cpool, [128, T], final_in[:], mdt.bfloat16, tag="finw")
            W_l = [load(cpool, [128, HH], W_in[li], mdt.bfloat16, tag=f"W{li}")
                   for li in range(L)]
            root_l = [load(cpool, [128, H], root_in[li], mdt.bfloat16,
                           tag=f"rw{li}") for li in range(L)]
            transh_l = [load(cpool, [128, H], transh_in[li], mdt.bfloat16,
                             tag=f"tt{li}") for li in range(L)]
            tbot_l = [load(cpool, [128, 4, H], tbot_in[li], mdt.bfloat16,
                           tag=f"tb{li}") for li in range(L)]
            gn_l = [load(cpool, [1, 4 * H], gn_in[li], tag=f"gn{li}")
                    for li in range(L)]

            # ---- one-hot blocks (host-built, graph-constant)
            onehot_sb = load(cpool, [128, B, 128], onehot_in[:], mdt.bfloat16,
                             tag="onehot")

            # ---- persistent tiles
            hA = hpool.tile([128, nub, H], mdt.float32)
            hB = hpool.tile([128, nub, H], mdt.float32)
            catTh = hpool.tile([128, nub, 128], mdt.bfloat16)
            catHC = hpool.tile([128, ngrp, 128], mdt.bfloat16)
            conv_sb = hpool.tile([128, nub, H], mdt.float32)
            hc_sb = hpool.tile([128, nub, H], mdt.float32)
            P_sb = hpool.tile([128, nub, H], mdt.float32)
            hsrc_sb = hpool.tile([128, B, H], mdt.bfloat16)
            hstage_sb = hpool.tile([128, nub, H], mdt.bfloat16)
            cd_sb = hpool.tile([128, 2 * H], mdt.float32)
            fcd_sb = hpool.tile([128, 2 * T], mdt.float32)
            fo_sb = hpool.tile([128, nub, T], mdt.float32)
            out_sb = hpool.tile([128, nub, T], mdt.float32)

            hstage_dram = dram.tile([shard_pad, H], mdt.bfloat16)
            n_total = shard_pad * NCORES
            htable_l = [dram.tile([n_total, H], mdt.bfloat16, addr_space="Shared",
                                  tag=f"htable{li}", name=f"htable{li}") for li in range(L)]
            st_in = dram.tile([1, 2 * H], mdt.float32)
            st_out_l = [dram.tile([1, 2 * H], mdt.float32, addr_space="Shared",
                                  tag=f"stout{li}", name=f"stout{li}") for li in range(L)]
            fst_in = dram.tile([1, 2 * T], mdt.float32)
            fst_out = dram.tile([1, 2 * T], mdt.float32, addr_space="Shared")

            hstage_v = hstage_dram[:].rearrange("(u p) f -> p u f", p=128)

            def cd_rows(crow, srow, gnrow, width, tag):
                """crow[0:w] = C = rstd*w ; crow[w:2w] = D = b - ms*mean*C.
                srow = [msq (0:w), mean (w:2w)] (1/n folded into mask).
                gnrow = [w, b, ms, ms*(2-ms)]."""
                gw = gnrow[:, 0:width]
                gb = gnrow[:, width : 2 * width]
                gms = gnrow[:, 2 * width : 3 * width]
                gco = gnrow[:, 3 * width : 4 * width]
                mean = srow[:, width : 2 * width]
                t = rpool.tile([1, width], mdt.float32, tag=tag)
                nc.vector.tensor_mul(t[:], mean, mean)
                nc.vector.tensor_mul(t[:], t[:], gco)
                nc.vector.tensor_sub(t[:], srow[:, 0:width], t[:])
                nc.vector.tensor_scalar_add(t[:], t[:], EPS)
                nc.vector.reciprocal(t[:], t[:])
                nc.scalar.activation(t[:], t[:], AF.Sqrt)
                nc.vector.tensor_mul(crow[:, 0:width], t[:], gw)
                nc.vector.tensor_mul(t[:], mean, gms)
                nc.vector.tensor_mul(t[:], t[:], crow[:, 0:width])
                nc.vector.scalar_tensor_tensor(
                    crow[:, width : 2 * width], t[:], -1.0, gb,
                    op0=ALU.mult, op1=ALU.add,
                )

            def transpose_h_to(hsrc_tile, dst_tile):
                """dst_tile[:H, w, :] (bf16) = hsrc_tile[:, w, :].T per window."""
                for w in range(nub):
                    tp = ps_pre.tile([H, 128], mdt.float32, tag="pre")
                    nc.tensor.transpose(tp[:], hsrc_tile[:, w, :], ident_sb[:])
                    nc.scalar.activation(dst_tile[0:H, w, :], tp[:], AF.Copy)

            # ============ layer 0: h0 = x @ init_W + b ============
            for u in range(nub):
                p = ps_sm.tile([128, H], mdt.float32, tag="sm")
                nc.tensor.matmul(p[:], xT_sb[:, u * 128 : (u + 1) * 128],
                                 init_sb[:], start=True, stop=True)
                nc.scalar.activation(hA[:, u, :], p[:], AF.Copy)
            nc.vector.tensor_copy(hstage_sb[:], hA[:])
            for p0 in range(H, 128, 32):
                nc.vector.memset(catTh[p0 : p0 + 32, :, :], 0.0)
            nc.vector.memset(catTh[H : H + 1, :, :], 1.0)
            transpose_h_to(hA, catTh)

            hcur, hnxt = hA, hB
            for li in range(L):
                # ---- stage h + AllGather
                nc.sync.dma_start(hstage_v, hstage_sb[:])
                htable_dram = htable_l[li]
                nc.gpsimd.collective_compute(
                    "AllGather", ALU.bypass, replica_groups=rg,
                    ins=[hstage_dram.opt()], outs=[htable_dram.opt()],
                )

                # ---- P = h @ trans_top + trans_b (overlaps AllGather)
                for w in range(nub):
                    pp = ps_sm.tile([128, H], mdt.float32, tag="sm")
                    nc.tensor.matmul(pp[:], catTh[:, w, :], transh_l[li][:],
                                     start=True, stop=True)
                    nc.scalar.activation(P_sb[:, w, :], pp[:], AF.Copy)

                # ---- gather h[src] per block
                for b in range(B):
                    nc.gpsimd.indirect_dma_start(
                        out=hsrc_sb[:, b, :],
                        out_offset=None,
                        in_=htable_dram[:],
                        in_offset=bass.IndirectOffsetOnAxis(
                            ap=idx_sb[:, b : b + 1], axis=0
                        ),
                    )

                # ---- edge phase
                agg = None
                for b in range(B):
                    w = block_win[b]
                    first = b == 0 or block_win[b - 1] != w
                    last = b == B - 1 or block_win[b + 1] != w

                    pre = ps_pre.tile([128, HH], mdt.float32, tag="pre")
                    a_sl = attrT_sb[:, b * 128 : (b + 1) * 128]
                    nc.tensor.matmul(pre[:, 0:512], a_sl, W_l[li][:, 0:512],
                                     start=True, stop=True)
                    nc.tensor.matmul(pre[:, 512:HH], a_sl, W_l[li][:, 512:HH],
                                     start=True, stop=True)

                    tmp = tmppool.tile([128, HH], mdt.bfloat16, tag="tmp")
                    if b % 4 == 3:
                        nc.vector.scalar_tensor_tensor(
                            tmp[:].rearrange("p (o i) -> p o i", o=H, i=H),
                            pre[:].rearrange("p (o i) -> p o i", o=H, i=H),
                            0.0,
                            hsrc_sb[:, b, :].unsqueeze(1)
                            .broadcast_to([128, H, H]),
                            op0=ALU.max, op1=ALU.mult,
                        )
                    else:
                        ew = tmppool.tile([128, HH], mdt.bfloat16, tag="ew")
                        nc.scalar.activation(ew[:], pre[:], AF.Relu)
                        nc.vector.tensor_tensor(
                            tmp[:].rearrange("p (o i) -> p o i", o=H, i=H),
                            ew[:].rearrange("p (o i) -> p o i", o=H, i=H),
                            hsrc_sb[:, b, :].unsqueeze(1)
                            .broadcast_to([128, H, H]),
                            op=ALU.mult,
                        )

                    # plain scatter into aggI; root+bias ride on the
                    # i=0 columns (strided PSUM write, N=32)
                    if first:
                        aggI = ps_aggi.tile([128, HH], mdt.float32, tag="aggI")
                    nc.tensor.matmul(aggI[:, 0:512], onehot_sb[:, b, :],
                                     tmp[:, 0:512], start=first, stop=False,
                                     skip_group_check=True)
                    nc.tensor.matmul(aggI[:, 512:HH], onehot_sb[:, b, :],
                                     tmp[:, 512:HH], start=first, stop=False,
                                     skip_group_check=True)
                    if first:
                        aggI_v = aggI[:].rearrange("p (o i) -> p o i", o=H, i=H)
                        for c in range(2):
                            nc.tensor.matmul(
                                aggI_v[:, c * 16 : (c + 1) * 16, 0:1],
                                catTh[:, w, :],
                                root_l[li][:, c * 16 : (c + 1) * 16],
                                start=False, stop=False, skip_group_check=True)

                    if last:
                        nc.vector.tensor_reduce(
                            conv_sb[:, w, :],
                            aggI[:].rearrange("p (o i) -> p o i", o=H, i=H),
                            axis=mybir.AxisListType.X, op=ALU.add,
                        )
                        csq = tmppool.tile([128, H], mdt.float32, tag="csq")
                        nc.vector.tensor_mul(csq[:], conv_sb[:, w, :],
                                             conv_sb[:, w, :])
                        if w == 0:
                            stp = ps_sm.tile([1, 2 * H], mdt.float32,
                                             tag="sm", name=f"stp{li}")
                        nc.tensor.matmul(stp[:, 0:H], mask_sb[:, w : w + 1],
                                         csq[:], start=(w == 0), stop=False,
                                         skip_group_check=True)
                        nc.tensor.matmul(stp[:, H : 2 * H],
                                         mask_sb[:, w : w + 1],
                                         conv_sb[:, w, :], start=False,
                                         stop=(w == nub - 1),
                                         skip_group_check=True)

                # ---- stats AllReduce
                stats_row = rpool.tile([1, 2 * H], mdt.float32, tag="srow")
                nc.vector.tensor_copy(stats_row[:], stp[:])
                nc.sync.dma_start(st_in[:], stats_row[:])
                st_out = st_out_l[li]
                nc.gpsimd.collective_compute(
                    "AllReduce", ALU.add, replica_groups=rg,
                    ins=[st_in.opt()], outs=[st_out.opt()],
                )
                srow2 = rpool.tile([1, 2 * H], mdt.float32, tag="srow2")
                nc.sync.dma_start(srow2[:], st_out[:])

                # ---- C/D rows + broadcast
                crow = rpool.tile([1, 2 * H], mdt.float32, tag="cdrow")
                cd_rows(crow, srow2, gn_l[li][:], H, "nrow")
                cd_ps = ps_sm.tile([128, 2 * H], mdt.float32, tag="sm")
                nc.tensor.matmul(cd_ps[:], onesr_sb[:], crow[:], start=True,
                                 stop=True)
                nc.scalar.activation(cd_sb[:], cd_ps[:], AF.Copy)

                # ---- hc = relu(conv*C + D) + h  (batched)
                cview = cd_sb[:, 0:H].unsqueeze(1).broadcast_to([128, nub, H])
                dview = cd_sb[:, H : 2 * H].unsqueeze(1) \
                    .broadcast_to([128, nub, H])
                nc.vector.tensor_tensor(hc_sb[:], conv_sb[:], cview,
                                        op=ALU.mult)
                nc.vector.tensor_tensor(hc_sb[:], hc_sb[:], dview, op=ALU.add)
                nc.vector.tensor_scalar_max(hc_sb[:], hc_sb[:], 0.0)

                # ---- hcT (batched 4-window transposes)
                for g in range(ngrp):
                    tp4 = ps_pre.tile([128, 128], mdt.float32, tag="pre")
                    nc.tensor.transpose(
                        tp4[:],
                        hc_sb[:, 4 * g : 4 * g + 4, :].rearrange(
                            "p u h -> p (u h)"),
                        ident_sb[:])
                    nc.scalar.activation(catHC[:, g, :], tp4[:], AF.Copy)

                # ---- transition: hnxt = relu(P + hc @ trans_bot)
                trp = ps_pre.tile([128, HH], mdt.float32, tag="pre")
                per_bank = 512 // H
                for w in range(nub):
                    nc.tensor.matmul(
                        trp[:, w * H : (w + 1) * H],
                        catHC[:, w // 4, :], tbot_l[li][:, w % 4, :],
                        start=(w % per_bank == 0),
                        stop=(w % per_bank == per_bank - 1 or w == nub - 1),
                        skip_group_check=True)
                trv = trp[:, 0 : nub * H].rearrange("p (u h) -> p u h", u=nub)
                nc.vector.tensor_tensor(hnxt[:], trv, P_sb[:], op=ALU.add)
                nc.vector.tensor_scalar_max(hnxt[:], hnxt[:], 0.0)
                nc.vector.tensor_copy(hstage_sb[:], hnxt[:])
                transpose_h_to(hnxt, catTh)

                hcur, hnxt = hnxt, hcur

            # ============ final ============
            fstp = ps_sm.tile([1, 2 * T], mdt.float32, tag="sm", name="fstp")
            fo_ps1 = ps_pre.tile([128, HH], mdt.float32, tag="pre",
                                 name="fops1")
            fo_ps2 = ps_pre.tile([128, HH], mdt.float32, tag="pre",
                                 name="fops2")
            nbank = 512 // T
            for w in range(nub):
                tgt = fo_ps1 if w < 16 else fo_ps2
                wo = w if w < 16 else w - 16
                nc.tensor.matmul(tgt[:, wo * T : (wo + 1) * T],
                                 catTh[:, w, :], final_sb[:],
                                 start=(wo % nbank == 0), stop=True,
                                 skip_group_check=True)
            nc.scalar.activation(
                fo_sb[:, 0:16, :].rearrange("p u f -> p (u f)"), fo_ps1[:],
                AF.Copy)
            nc.scalar.activation(
                fo_sb[:, 16:nub, :].rearrange("p u f -> p (u f)"),
                fo_ps2[:, 0 : (nub - 16) * T], AF.Copy)
            fsq = tmppool.tile([128, nub * T], mdt.float32, tag="fsq")
            nc.vector.tensor_mul(fsq[:], fo_sb[:].rearrange("p u f -> p (u f)"),
                                 fo_sb[:].rearrange("p u f -> p (u f)"))
            fsq_v = fsq[:].rearrange("p (u f) -> p u f", u=nub)
            for w in range(nub):
                nc.tensor.matmul(fstp[:, 0:T], mask_sb[:, w : w + 1],
                                 fsq_v[:, w, :], start=(w == 0), stop=False,
                                 skip_group_check=True)
                nc.tensor.matmul(fstp[:, T : 2 * T], mask_sb[:, w : w + 1],
                                 fo_sb[:, w, :], start=False,
                                 stop=(w == nub - 1), skip_group_check=True)

            fstats_row = rpool.tile([1, 2 * T], mdt.float32, tag="fsrow")
            nc.vector.tensor_copy(fstats_row[:], fstp[:])
            nc.sync.dma_start(fst_in[:], fstats_row[:])
            nc.gpsimd.collective_compute(
                "AllReduce", ALU.add, replica_groups=rg,
                ins=[fst_in.opt()], outs=[fst_out.opt()],
            )
            fsrow2 = rpool.tile([1, 2 * T], mdt.float32, tag="fsrow2")
            nc.sync.dma_start(fsrow2[:], fst_out[:])

            fcrow = rpool.tile([1, 2 * T], mdt.float32, tag="fcdrow")
            cd_rows(fcrow, fsrow2, fgn_sb[:], T, "frow")
            fcd_ps = ps_sm.tile([128, 2 * T], mdt.float32, tag="sm")
            nc.tensor.matmul(fcd_ps[:], onesr_sb[:], fcrow[:], start=True,
                             stop=True)
            nc.scalar.activation(fcd_sb[:], fcd_ps[:], AF.Copy)

            fcv = fcd_sb[:, 0:T].unsqueeze(1).broadcast_to([128, nub, T])
            fdv = fcd_sb[:, T : 2 * T].unsqueeze(1).broadcast_to([128, nub, T])
            nc.vector.tensor_tensor(out_sb[:], fo_sb[:], fcv, op=ALU.mult)
            nc.vector.tensor_tensor(out_sb[:], out_sb[:], fdv, op=ALU.add)
            nc.vector.tensor_scalar_max(out_sb[:], out_sb[:], 0.0)
            out_v = out_dram.rearrange("(u p) f -> p u f", p=128)
            nc.sync.dma_start(out_v, out_sb[:])

    nc.compile()
    return nc


def _dedup_ldweights(nc):
    """Remove PE LDWEIGHTS whose weights AP matches the immediately preceding
    LDWEIGHTS on the PE stream (bass emits one per matmul even when
    consecutive matmuls share the stationary). Waits/updates on a removed
    LDW migrate to the next PE matmul so gating semantics are preserved."""
    pe = mybir.EngineType.PE
    removed = 0
    for blk in nc.m.functions[0].blocks:
        insts = blk.instructions
        last_key = None
        pending = None  # sync_info carried from a removed LDW
        keep = []
        for inst in insts:
            if getattr(inst, "engine", None) != pe:
                keep.append(inst)
                continue
            if isinstance(inst, mybir.InstLdweights):
                w0 = inst.ins[0]
                key = (str(getattr(w0, "memref", "")), str(w0))
                if key == last_key:
                    si = inst.sync_info
                    if si is not None and (si.on_wait or si.on_update):
                        if pending is None:
                            pending = ([], [])
                        pending[0].extend(si.on_wait)
                        pending[1].extend(si.on_update)
                    removed += 1
                    continue
                last_key = key
                keep.append(inst)
            else:
                if isinstance(inst, mybir.InstMatmult) and pending is not None:
                    si = inst.sync_info
                    if si is None:
                        inst.sync_info = mybir.SyncInfo(
                            on_wait=pending[0], on_update=pending[1])
                    else:
                        inst.sync_info = mybir.SyncInfo(
                            on_wait=list(si.on_wait) + pending[0],
                            on_update=list(si.on_update) + pending[1])
                    pending = None
                keep.append(inst)
        if removed:
            blk.instructions.clear()
            for i in keep:
                blk.instructions.append(i)
    return removed


# ------------------------------------------------------------------ driver

_CACHE = {}


def kernel(**inputs) -> np.ndarray:
    in_maps, s, perms = prep_inputs(inputs)
    key = (s["N"], s["E"], s["B"], s["block_win"])
    if key not in _CACHE:
        _CACHE[key] = build_program(s)
    nc = _CACHE[key]

    from concourse.bass_utils import run_bass_kernel_spmd

    res = run_bass_kernel_spmd(nc, in_maps, core_ids=list(range(NCORES)))
    shard = s["shard"]
    outs = [res.results[c]["out"][perms[c][:shard]] for c in range(NCORES)]
    return np.concatenate(outs, axis=0).astype(np.float32)
